# revision 5
# baseline (speedup 1.0000x reference)
"""Trainium2 Bass kernel for BasicMoE.

Reference computation (N=8192 tokens, D=1024 in, O=1024 out, E=8 experts):
    gates = softmax(x @ Wg + bg)                        # [N, E]
    out   = sum_e gates[:, e] * (x @ We[e] + be[e])     # [N, O]

Strategy: data-parallel over tokens (1024 tokens/core, replicated weights),
with a mixed-precision decomposition that moves 8/9 of the matmul FLOPs to
fp8 DoubleRow (2 MACs/cell/cycle):

    out = x @ Wmean + g @ be                       (bf16 GEMMs, accurate)
        + sum_e (g_e - 1/8) * (x8 @ Wp8_e)         (fp8e4 DoubleRow GEMMs)

where Wmean = mean_e We, Wp_e = We - Wmean, and sum_e (g_e - 1/8) Wp_e
== sum_e g_e Wp_e exactly because sum_e Wp_e = 0. Quantization noise of
the fp8 GEMMs enters scaled by ||g - 1/8|| ~ 0.19 instead of ||g|| ~ 0.38,
which keeps the end-to-end rel err at ~1.75e-2 (measured against the
reference on the true inputs) under the 2e-2 gate.

Per-core schedule:
  W : ~88 throwaway identity matmuls while the input DMA streams in; they
      trip the PE HAM activity monitor so the real GEMMs start at 2.4 GHz
      instead of spending their first ~3.4us at 1.2 GHz.
  A : per token-tile: gating logits (PE) + softmax (ACT/DVE); also
      cc = (g - 1/8)/2^16 (the 2^16 removes the fp8 scale factors XS*WS).
  A2: transpose gates (PE) for the bias matmul.
  B0: acc = x @ Wmean + gT.T @ be  (bf16 PE; PSUM->SBUF copy on ACT).
  B : for e,t,j: psum = DR-matmul(x8, Wp8_e) over k-pairs;
      acc += psum * cc[:, e] (DVE scalar_tensor_tensor);
      last expert streams acc out to HBM on both DMA rings.

Data layouts are t-major / j-major so every DMA moves >=1KB per partition
line and arrives in exactly the order the compute consumes it.
"""

import numpy as np
import ml_dtypes

N_TOKENS = 8192
D = 1024   # in dim
O = 1024   # out dim
E = 8      # experts
NCORES = 8
NLOC = N_TOKENS // NCORES   # 1024 tokens per core
KT = D // 128               # 8 k-chunks
TT = NLOC // 128            # 8 token chunks
JT = O // 512               # 2 out chunks

BF16 = ml_dtypes.bfloat16
F8E4 = ml_dtypes.float8_e4m3   # IEEE e4m3: max normal 240, matches TRN fp8e4

XS = 32.0      # x fp8 scale (|x| < 5.2 -> < 166)
WS = 2048.0    # Wp fp8 scale (|Wp| < 0.051 -> < 105)
CINV = 1.0 / (XS * WS)

NWARM = 88     # HAM warm-up matmuls

_CACHE = {}


def _build():
    """Build + compile the per-core Bass graph (same graph on all 8 cores)."""
    import concourse.bass as bass
    import concourse.mybir as mybir
    import concourse.tile as tile
    from concourse import bacc
    from concourse.masks import make_identity

    dt = mybir.dt
    f32 = dt.float32
    bf16 = dt.bfloat16
    f8e4 = dt.float8e4
    Alu = mybir.AluOpType
    DR = mybir.MatmulPerfMode.DoubleRow
    Act = mybir.ActivationFunctionType

    nc = bacc.Bacc(
        "TRN2",
        target_bir_lowering=False,
        debug=False,
        enable_asserts=False,
        num_devices=NCORES,
    )

    # t-major x: xt[p, t*D + k*128 + c] = x[t*128 + c, k*128 + p]
    xt_d = nc.dram_tensor("xt", [128, TT * D], bf16, kind="ExternalInput").ap()
    xt8_d = nc.dram_tensor("xt8", [128, TT * D], f8e4, kind="ExternalInput").ap()
    # j-major weights: w[p, j*KT*512 + k*512 + c] = W[k*128 + p, j*512 + c]
    wm_d = nc.dram_tensor("Wmp", [128, JT * KT * 512], bf16, kind="ExternalInput").ap()
    wp8_d = nc.dram_tensor(
        "Wp8", [E, 128, JT * KT * 512], f8e4, kind="ExternalInput"
    ).ap()
    be_d = nc.dram_tensor("bep", [E, O], bf16, kind="ExternalInput").ap()
    wg_d = nc.dram_tensor("Wgp", [128, KT * E], bf16, kind="ExternalInput").ap()
    bg_d = nc.dram_tensor("bgp", [1, E], bf16, kind="ExternalInput").ap()
    out_d = nc.dram_tensor("out", [NLOC, O], f32, kind="ExternalOutput").ap()

    with tile.TileContext(nc) as tc:
        with (
            tc.tile_pool(name="const", bufs=1) as cpool,
            tc.tile_pool(name="xp", bufs=1) as xpool,
            tc.tile_pool(name="wp", bufs=E) as wpool,
            tc.tile_pool(name="ap", bufs=1) as apool,
            tc.tile_pool(name="gp", bufs=1) as gpool,
        ):
            ident = cpool.tile([128, 128], bf16)
            make_identity(nc, ident[:])
            ones = cpool.tile([1, 128], bf16)
            nc.gpsimd.memset(ones[:], 1.0)

            # Tiny gating/bias constants first on the sync ring (~20KB).
            wg_sb = cpool.tile([128, KT * E], bf16)
            nc.sync.dma_start(wg_sb[:], wg_d)
            bg_sb = cpool.tile([1, E], bf16)
            nc.sync.dma_start(bg_sb[:], bg_d)
            be_sb = cpool.tile([E, O], bf16)
            nc.sync.dma_start(be_sb[:], be_d)

            # x t-major, alternating rings by tile parity so gating can start
            # on tile 0 while the rest stream.
            xt = xpool.tile([128, TT * D], bf16)
            for t in range(TT):
                eng = nc.sync if t % 2 == 0 else nc.scalar
                eng.dma_start(
                    xt[:, t * D : (t + 1) * D], xt_d[:, t * D : (t + 1) * D]
                )
            # Wmean j-halves split across the rings (mean GEMM is next).
            wm = xpool.tile([128, JT, KT, 512], bf16)
            wm_dv = wm_d.rearrange("p (j r) -> p j r", j=JT)
            for j in range(JT):
                eng = nc.sync if j == 0 else nc.scalar
                eng.dma_start(wm[:, j].rearrange("p k c -> p (k c)"), wm_dv[:, j])
            # x8 (needed only when phase B starts).
            xt8 = xpool.tile([128, TT, KT, 128], f8e4)
            nc.scalar.dma_start(
                xt8[:].rearrange("p t k c -> p (t k c)"), xt8_d
            )
            # Expert fp8 weights, j-halves across rings, expert-major.
            wp8_tiles = []
            wp8_dv = wp8_d.rearrange("e p (j r) -> e p j r", j=JT)
            for e in range(E):
                w8 = wpool.tile([128, JT, KT, 512], f8e4, tag="wp8", name=f"wp8_{e}")
                for j in range(JT):
                    eng = nc.sync if j == 0 else nc.scalar
                    eng.dma_start(
                        w8[:, j].rearrange("p k c -> p (k c)"), wp8_dv[e, :, j]
                    )
                wp8_tiles.append(w8)

            acc = apool.tile([128, TT * O], f32)

            g_f32 = gpool.tile([128, TT * E], f32)
            cc_f32 = gpool.tile([128, TT * E], f32)
            g_bf = gpool.tile([128, TT * E], bf16)
            gT = gpool.tile([E, NLOC], bf16)
            negm = gpool.tile([128, TT], f32)
            ssum = gpool.tile([128, TT], f32)
            rec = gpool.tile([128, TT], f32)

            def xt_tile(k, t):
                c = t * D + k * 128
                return xt[:, c : c + 128]

            # ---- Phase W: HAM warm-up on junk matmuls ---------------------
            with tc.tile_pool(name="psW", bufs=2, space="PSUM") as psW:
                wj = [psW.tile([128, 128], f32, tag="wj", name=f"wj{i}") for i in range(2)]
                for i in range(NWARM):
                    nc.tensor.matmul(
                        wj[i % 2][:], ident[:], ident[:], start=True, stop=True
                    )

            # ---- Phase A: gating logits + softmax --------------------------
            with tc.tile_pool(name="psA", bufs=2, space="PSUM") as psA:
                for t in range(TT):
                    zg = psA.tile([128, E], f32, tag="zg")
                    for k in range(KT):
                        nc.tensor.matmul(
                            zg[:],
                            xt_tile(k, t),
                            wg_sb[:, k * E : (k + 1) * E],
                            start=(k == 0),
                            stop=False,
                        )
                    # + bg (rank-1: ones[1,128].T @ bg[1,E])
                    nc.tensor.matmul(zg[:], ones[:], bg_sb[:], start=False, stop=True)

                    nm = negm[:, t : t + 1]
                    nc.vector.tensor_reduce(
                        nm, zg[:], axis=mybir.AxisListType.X, op=Alu.max, negate=True
                    )
                    gs = g_f32[:, t * E : (t + 1) * E]
                    nc.scalar.activation(
                        gs,
                        zg[:],
                        Act.Exp,
                        bias=nm,
                        scale=1.0,
                        accum_out=ssum[:, t : t + 1],
                    )
                    nc.vector.reciprocal(rec[:, t : t + 1], ssum[:, t : t + 1])
                    nc.vector.tensor_scalar_mul(gs, gs, rec[:, t : t + 1])
                    # cc = (g - 1/8) * CINV -- fp8-scale-corrected centered gates
                    nc.vector.tensor_scalar(
                        cc_f32[:, t * E : (t + 1) * E],
                        gs,
                        -0.125,
                        CINV,
                        op0=Alu.add,
                        op1=Alu.mult,
                    )
                    nc.gpsimd.tensor_copy(g_bf[:, t * E : (t + 1) * E], gs)

            # ---- Phase A2: transpose gates for the bias matmul -------------
            with tc.tile_pool(name="psC", bufs=1, space="PSUM") as psC:
                for t in range(TT):
                    trp = psC.tile([E, 128], bf16, tag="tr")
                    nc.tensor.transpose(
                        trp[:], g_bf[:, t * E : (t + 1) * E], ident[:]
                    )
                    nc.vector.tensor_copy(gT[:, t * 128 : (t + 1) * 128], trp[:])

            # ---- Phase B0: mean GEMM + bias -------------------------------
            # acc[t, j] = sum_k x_t @ Wmean[k, j] + gT_t.T @ be[:, j]
            with tc.tile_pool(name="psM", bufs=4, space="PSUM") as psM:
                for t in range(TT):
                    for j in range(JT):
                        pm = psM.tile([128, 512], f32, tag="pm")
                        for k in range(KT):
                            nc.tensor.matmul(
                                pm[:],
                                xt_tile(k, t),
                                wm[:, j, k, :],
                                start=(k == 0),
                                stop=False,
                            )
                        nc.tensor.matmul(
                            pm[:],
                            gT[:, t * 128 : (t + 1) * 128],
                            be_sb[:, j * 512 : (j + 1) * 512],
                            start=False,
                            stop=True,
                        )
                        # PSUM -> SBUF on the ACT engine; keeps DVE free for
                        # the correction-phase scalar_tensor_tensors.
                        nc.scalar.activation(
                            acc[:, t * O + j * 512 : t * O + (j + 1) * 512],
                            pm[:],
                            Act.Copy,
                        )

            # ---- Phase B: fp8 DoubleRow correction GEMMs ------------------
            # psum[t,j] = sum_{k-pairs} DR(x8, Wp8_e); acc += psum * cc[:, e]
            with tc.tile_pool(name="psB", bufs=6, space="PSUM") as psB:
                for e in range(E):
                    w8 = wp8_tiles[e]
                    last = e == E - 1
                    for t in range(TT):
                        ccol = cc_f32[:, t * E + e : t * E + e + 1]
                        for j in range(JT):
                            ps = psB.tile([128, 512], f32, tag="mm")
                            for k2 in range(KT // 2):
                                nc.tensor.matmul(
                                    ps[:],
                                    xt8[:, t, 2 * k2 : 2 * k2 + 2, :],
                                    w8[:, j, 2 * k2 : 2 * k2 + 2, :],
                                    start=(k2 == 0),
                                    stop=(k2 == KT // 2 - 1),
                                    perf_mode=DR,
                                )
                            a_sl = acc[:, t * O + j * 512 : t * O + (j + 1) * 512]
                            nc.vector.scalar_tensor_tensor(
                                a_sl, ps[:], ccol, a_sl,
                                op0=Alu.mult, op1=Alu.add,
                            )
                            if last:
                                eng = nc.sync if j == 0 else nc.scalar
                                eng.dma_start(
                                    out_d[
                                        t * 128 : (t + 1) * 128,
                                        j * 512 : (j + 1) * 512,
                                    ],
                                    a_sl,
                                )

    nc.compile()
    return nc


def _get_nc():
    if "nc" not in _CACHE:
        _CACHE["nc"] = _build()
    return _CACHE["nc"]


def _pack_inputs(x, We, be, Wg, bg):
    """Host-side packing: shard + pre-transpose + cast to bf16/fp8."""
    x = np.asarray(x, dtype=np.float32)
    We = np.asarray(We, dtype=np.float32)
    be = np.asarray(be, dtype=np.float32)
    Wg = np.asarray(Wg, dtype=np.float32)
    bg = np.asarray(bg, dtype=np.float32)

    Wmean = We.mean(axis=0)
    Wp = We - Wmean[None]

    def ptrans_j(w):  # [D, O] -> [128, JT*KT*512], [p, j, k, c] = w[k*128+p, j*512+c]
        return np.ascontiguousarray(
            w.reshape(KT, 128, JT, 512).transpose(1, 2, 0, 3).reshape(128, -1)
        )

    wm_p = ptrans_j(Wmean).astype(BF16)
    wp8_p = np.stack(
        [np.clip(ptrans_j(Wp[e]) * WS, -240, 240).astype(F8E4) for e in range(E)]
    )
    be_p = be.astype(BF16)
    wg_p = np.ascontiguousarray(
        Wg.reshape(KT, 128, E).transpose(1, 0, 2).reshape(128, KT * E)
    ).astype(BF16)
    bg_p = bg.reshape(1, E).astype(BF16)

    in_maps = []
    for i in range(NCORES):
        xs = x[i * NLOC : (i + 1) * NLOC]          # [NLOC, D]
        # xt[p, t*D + k*128 + c] = xs[t*128+c, k*128+p]
        xt_f = np.ascontiguousarray(
            xs.T.reshape(KT, 128, TT, 128).transpose(1, 2, 0, 3).reshape(128, TT * D)
        )
        xt = xt_f.astype(BF16)
        xt8 = np.clip(xt_f * XS, -240, 240).astype(F8E4)
        in_maps.append(
            {
                "xt": xt,
                "xt8": xt8,
                "Wmp": wm_p,
                "Wp8": wp8_p,
                "bep": be_p,
                "Wgp": wg_p,
                "bgp": bg_p,
            }
        )
    return in_maps


def _run(inputs, trace=False):
    """Returns (y_full, BassKernelResults)."""
    from concourse.bass_utils import run_bass_kernel_spmd

    nc = _get_nc()
    in_maps = _pack_inputs(**inputs)
    res = run_bass_kernel_spmd(
        nc, in_maps, core_ids=list(range(NCORES)), trace=trace
    )
    y = np.concatenate(
        [res.results[i]["out"] for i in range(NCORES)], axis=0
    ).astype(np.float32)
    return y, res


def kernel(**inputs):
    y, _ = _run(inputs, trace=False)
    return y


# revision 6
# speedup vs baseline: 1.0850x; 1.0850x over previous
"""Trainium2 Bass kernel for BasicMoE.

Reference computation (N=8192 tokens, D=1024 in, O=1024 out, E=8 experts):
    gates = softmax(x @ Wg + bg)                        # [N, E]
    out   = sum_e gates[:, e] * (x @ We[e] + be[e])     # [N, O]

Strategy: data-parallel over tokens (1024 tokens/core, replicated weights),
with a mixed-precision decomposition that moves 8/9 of the matmul FLOPs to
fp8 DoubleRow (2 MACs/cell/cycle):

    out = x @ Wmean + g @ be                       (bf16 GEMMs, accurate)
        + sum_e (g_e - 1/8) * (x8 @ Wp8_e)         (fp8e4 DoubleRow GEMMs)

where Wmean = mean_e We, Wp_e = We - Wmean, and sum_e (g_e - 1/8) Wp_e
== sum_e g_e Wp_e exactly because sum_e Wp_e = 0. Quantization noise of
the fp8 GEMMs enters scaled by ||g - 1/8|| ~ 0.19 instead of ||g|| ~ 0.38,
which keeps the end-to-end rel err at ~1.75e-2 (measured against the
reference on the true inputs) under the 2e-2 gate.

Per-core schedule:
  W : ~88 throwaway identity matmuls while the input DMA streams in; they
      trip the PE HAM activity monitor so the real GEMMs start at 2.4 GHz
      instead of spending their first ~3.4us at 1.2 GHz.
  A : gating logits computed TRANSPOSED -- zT[e, n] = sum_k Wg_k.T @ x_k
      as 16 N=512 matmuls (tiny-N matmuls are issue-latency-bound: the
      direct [n,e] form needs 72 matmuls at N=8). ACT adds bg (per-
      partition bias) while copying PSUM->SBUF; PE transposes each token
      tile back to [n, e]; softmax per tile on DVE/ACT; also
      cc = (g - 1/8)/2^16 (the 2^16 removes the fp8 scale factors XS*WS).
  A2: transpose gates (PE) for the bias matmul.
  B0: acc = x @ Wmean + gT.T @ be  (bf16 PE; PSUM->SBUF copy on ACT).
  B : for e,t,j: psum = DR-matmul(x8, Wp8_e) over k-pairs;
      acc += psum * cc[:, e] (DVE scalar_tensor_tensor);
      last expert streams acc out to HBM on both DMA rings.

All input DMAs are issued on the sync-engine HWDGE ring in consumption
order (one ring saturates HBM; issuing bulk DMAs from the ACT/DVE queues
blocks their compute behind the transfers -- measured 17us stall).
Data layouts are t-major / j-major so every DMA moves >=1KB per partition
line and arrives in exactly the order the compute consumes it.
"""

import numpy as np
import ml_dtypes

N_TOKENS = 8192
D = 1024   # in dim
O = 1024   # out dim
E = 8      # experts
NCORES = 8
NLOC = N_TOKENS // NCORES   # 1024 tokens per core
KT = D // 128               # 8 k-chunks
TT = NLOC // 128            # 8 token chunks
JT = O // 512               # 2 out chunks

BF16 = ml_dtypes.bfloat16
F8E4 = ml_dtypes.float8_e4m3   # IEEE e4m3: max normal 240, matches TRN fp8e4

XS = 32.0      # x fp8 scale (|x| < 5.2 -> < 166)
WS = 2048.0    # Wp fp8 scale (|Wp| < 0.051 -> < 105)
CINV = 1.0 / (XS * WS)

NWARM = 88     # HAM warm-up matmuls

_CACHE = {}


def _build():
    """Build + compile the per-core Bass graph (same graph on all 8 cores)."""
    import concourse.bass as bass
    import concourse.mybir as mybir
    import concourse.tile as tile
    from concourse import bacc
    from concourse.masks import make_identity

    dt = mybir.dt
    f32 = dt.float32
    bf16 = dt.bfloat16
    f8e4 = dt.float8e4
    Alu = mybir.AluOpType
    DR = mybir.MatmulPerfMode.DoubleRow
    Act = mybir.ActivationFunctionType

    nc = bacc.Bacc(
        "TRN2",
        target_bir_lowering=False,
        debug=False,
        enable_asserts=False,
        num_devices=NCORES,
    )

    # t-major x: xt[p, t*D + k*128 + c] = x[t*128 + c, k*128 + p]
    xt_d = nc.dram_tensor("xt", [128, TT * D], bf16, kind="ExternalInput").ap()
    xt8_d = nc.dram_tensor("xt8", [128, TT * D], f8e4, kind="ExternalInput").ap()
    # j-major weights: w[p, j*KT*512 + k*512 + c] = W[k*128 + p, j*512 + c]
    wm_d = nc.dram_tensor("Wmp", [128, JT * KT * 512], bf16, kind="ExternalInput").ap()
    wp8_d = nc.dram_tensor(
        "Wp8", [E, 128, JT * KT * 512], f8e4, kind="ExternalInput"
    ).ap()
    be_d = nc.dram_tensor("bep", [E, O], bf16, kind="ExternalInput").ap()
    wg_d = nc.dram_tensor("Wgp", [128, KT * E], bf16, kind="ExternalInput").ap()
    bg_d = nc.dram_tensor("bgp", [E, 1], f32, kind="ExternalInput").ap()
    out_d = nc.dram_tensor("out", [NLOC, O], f32, kind="ExternalOutput").ap()

    with tile.TileContext(nc) as tc:
        with (
            tc.tile_pool(name="const", bufs=1) as cpool,
            tc.tile_pool(name="xp", bufs=1) as xpool,
            tc.tile_pool(name="wp", bufs=E) as wpool,
            tc.tile_pool(name="ap", bufs=1) as apool,
            tc.tile_pool(name="gp", bufs=1) as gpool,
        ):
            ident = cpool.tile([128, 128], bf16)
            make_identity(nc, ident[:])

            # All input DMAs on the sync HWDGE ring, in consumption order.
            wg_sb = cpool.tile([128, KT * E], bf16)
            nc.sync.dma_start(wg_sb[:], wg_d)
            bg_sb = cpool.tile([E, 1], f32)
            nc.sync.dma_start(bg_sb[:], bg_d)
            be_sb = cpool.tile([E, O], bf16)
            nc.sync.dma_start(be_sb[:], be_d)

            xt = xpool.tile([128, TT * D], bf16)
            for t in range(TT):
                nc.sync.dma_start(
                    xt[:, t * D : (t + 1) * D], xt_d[:, t * D : (t + 1) * D]
                )
            wm = xpool.tile([128, JT, KT, 512], bf16)
            wm_dv = wm_d.rearrange("p (j r) -> p j r", j=JT)
            for j in range(JT):
                nc.sync.dma_start(wm[:, j].rearrange("p k c -> p (k c)"), wm_dv[:, j])
            xt8 = xpool.tile([128, TT, KT, 128], f8e4)
            nc.sync.dma_start(xt8[:].rearrange("p t k c -> p (t k c)"), xt8_d)
            wp8_tiles = []
            wp8_dv = wp8_d.rearrange("e p (j r) -> e p j r", j=JT)
            for e in range(E):
                w8 = wpool.tile([128, JT, KT, 512], f8e4, tag="wp8", name=f"wp8_{e}")
                for j in range(JT):
                    nc.sync.dma_start(
                        w8[:, j].rearrange("p k c -> p (k c)"), wp8_dv[e, :, j]
                    )
                wp8_tiles.append(w8)

            acc = apool.tile([128, TT * O], f32)

            zT_sb = gpool.tile([E, NLOC], bf16)
            g_f32 = gpool.tile([128, TT * E], f32)
            cc_f32 = gpool.tile([128, TT * E], f32)
            g_bf = gpool.tile([128, TT * E], bf16)
            gT = gpool.tile([E, NLOC], bf16)
            negm = gpool.tile([128, TT], f32)
            ssum = gpool.tile([128, TT], f32)
            rec = gpool.tile([128, TT], f32)

            xt_4d = xt.rearrange("p (t k c) -> p t k c", t=TT, k=KT)

            def xt_tile(k, t):
                c = t * D + k * 128
                return xt[:, c : c + 128]

            # ---- Phase W: HAM warm-up on junk matmuls ---------------------
            with tc.tile_pool(name="psW", bufs=2, space="PSUM") as psW:
                wj = [
                    psW.tile([128, 128], f32, tag="wj", name=f"wj{i}")
                    for i in range(2)
                ]
                for i in range(NWARM):
                    nc.tensor.matmul(
                        wj[i % 2][:], ident[:], ident[:], start=True, stop=True
                    )

            # ---- Phase A: gating logits (transposed) + softmax -------------
            with (
                tc.tile_pool(name="psZ", bufs=2, space="PSUM") as psZ,
                tc.tile_pool(name="psA", bufs=4, space="PSUM") as psA,
            ):
                th = TT // 2
                for h in range(2):  # token halves of 512
                    ztp = psZ.tile([E, 512], f32, tag="zt")
                    for k in range(KT):
                        nc.tensor.matmul(
                            ztp[:],
                            wg_sb[:, k * E : (k + 1) * E],
                            xt_4d[:, h * th : (h + 1) * th, k, :],
                            start=(k == 0),
                            stop=(k == KT - 1),
                        )
                    # + bg (per-partition bias) while copying PSUM -> SBUF
                    nc.scalar.activation(
                        zT_sb[:, h * 512 : (h + 1) * 512],
                        ztp[:],
                        Act.Identity,
                        bias=bg_sb[:, 0:1],
                        scale=1.0,
                    )
                for t in range(TT):
                    zg = psA.tile([128, E], bf16, tag="zg")
                    nc.tensor.transpose(
                        zg[:], zT_sb[:, t * 128 : (t + 1) * 128], ident[:E, :E]
                    )
                    nm = negm[:, t : t + 1]
                    nc.vector.tensor_reduce(
                        nm, zg[:], axis=mybir.AxisListType.X, op=Alu.max, negate=True
                    )
                    gs = g_f32[:, t * E : (t + 1) * E]
                    nc.scalar.activation(
                        gs,
                        zg[:],
                        Act.Exp,
                        bias=nm,
                        scale=1.0,
                        accum_out=ssum[:, t : t + 1],
                    )
                    nc.vector.reciprocal(rec[:, t : t + 1], ssum[:, t : t + 1])
                    nc.vector.tensor_scalar_mul(gs, gs, rec[:, t : t + 1])
                    # cc = (g - 1/8) * CINV -- fp8-scale-corrected centered gates
                    nc.vector.tensor_scalar(
                        cc_f32[:, t * E : (t + 1) * E],
                        gs,
                        -0.125,
                        CINV,
                        op0=Alu.add,
                        op1=Alu.mult,
                    )
                    nc.gpsimd.tensor_copy(g_bf[:, t * E : (t + 1) * E], gs)

            # ---- Phase A2: transpose gates for the bias matmul -------------
            with tc.tile_pool(name="psC", bufs=2, space="PSUM") as psC:
                for t in range(TT):
                    trp = psC.tile([E, 128], bf16, tag="tr")
                    nc.tensor.transpose(
                        trp[:], g_bf[:, t * E : (t + 1) * E], ident[:]
                    )
                    nc.vector.tensor_copy(gT[:, t * 128 : (t + 1) * 128], trp[:])

            # ---- Phase B0: mean GEMM + bias -------------------------------
            # acc[t, j] = sum_k x_t @ Wmean[k, j] + gT_t.T @ be[:, j]
            with tc.tile_pool(name="psM", bufs=4, space="PSUM") as psM:
                for t in range(TT):
                    for j in range(JT):
                        pm = psM.tile([128, 512], f32, tag="pm")
                        for k in range(KT):
                            nc.tensor.matmul(
                                pm[:],
                                xt_tile(k, t),
                                wm[:, j, k, :],
                                start=(k == 0),
                                stop=False,
                            )
                        nc.tensor.matmul(
                            pm[:],
                            gT[:, t * 128 : (t + 1) * 128],
                            be_sb[:, j * 512 : (j + 1) * 512],
                            start=False,
                            stop=True,
                        )
                        # PSUM -> SBUF on the ACT engine; keeps DVE free for
                        # the correction-phase scalar_tensor_tensors.
                        nc.scalar.activation(
                            acc[:, t * O + j * 512 : t * O + (j + 1) * 512],
                            pm[:],
                            Act.Copy,
                        )

            # ---- Phase B: fp8 DoubleRow correction GEMMs ------------------
            # psum[t,j] = sum_{k-pairs} DR(x8, Wp8_e); acc += psum * cc[:, e]
            with tc.tile_pool(name="psB", bufs=6, space="PSUM") as psB:
                for e in range(E):
                    w8 = wp8_tiles[e]
                    last = e == E - 1
                    for t in range(TT):
                        ccol = cc_f32[:, t * E + e : t * E + e + 1]
                        for j in range(JT):
                            ps = psB.tile([128, 512], f32, tag="mm")
                            for k2 in range(KT // 2):
                                nc.tensor.matmul(
                                    ps[:],
                                    xt8[:, t, 2 * k2 : 2 * k2 + 2, :],
                                    w8[:, j, 2 * k2 : 2 * k2 + 2, :],
                                    start=(k2 == 0),
                                    stop=(k2 == KT // 2 - 1),
                                    perf_mode=DR,
                                )
                            a_sl = acc[:, t * O + j * 512 : t * O + (j + 1) * 512]
                            nc.vector.scalar_tensor_tensor(
                                a_sl, ps[:], ccol, a_sl,
                                op0=Alu.mult, op1=Alu.add,
                            )
                            if last:
                                eng = nc.sync if j == 0 else nc.scalar
                                eng.dma_start(
                                    out_d[
                                        t * 128 : (t + 1) * 128,
                                        j * 512 : (j + 1) * 512,
                                    ],
                                    a_sl,
                                )

    nc.compile()
    return nc


def _get_nc():
    if "nc" not in _CACHE:
        _CACHE["nc"] = _build()
    return _CACHE["nc"]


def _pack_inputs(x, We, be, Wg, bg):
    """Host-side packing: shard + pre-transpose + cast to bf16/fp8."""
    x = np.asarray(x, dtype=np.float32)
    We = np.asarray(We, dtype=np.float32)
    be = np.asarray(be, dtype=np.float32)
    Wg = np.asarray(Wg, dtype=np.float32)
    bg = np.asarray(bg, dtype=np.float32)

    Wmean = We.mean(axis=0)
    Wp = We - Wmean[None]

    def ptrans_j(w):  # [D, O] -> [128, JT*KT*512], [p, j, k, c] = w[k*128+p, j*512+c]
        return np.ascontiguousarray(
            w.reshape(KT, 128, JT, 512).transpose(1, 2, 0, 3).reshape(128, -1)
        )

    wm_p = ptrans_j(Wmean).astype(BF16)
    wp8_p = np.stack(
        [np.clip(ptrans_j(Wp[e]) * WS, -240, 240).astype(F8E4) for e in range(E)]
    )
    be_p = be.astype(BF16)
    wg_p = np.ascontiguousarray(
        Wg.reshape(KT, 128, E).transpose(1, 0, 2).reshape(128, KT * E)
    ).astype(BF16)
    bg_p = bg.reshape(E, 1).astype(np.float32)

    in_maps = []
    for i in range(NCORES):
        xs = x[i * NLOC : (i + 1) * NLOC]          # [NLOC, D]
        # xt[p, t*D + k*128 + c] = xs[t*128+c, k*128+p]
        xt_f = np.ascontiguousarray(
            xs.T.reshape(KT, 128, TT, 128).transpose(1, 2, 0, 3).reshape(128, TT * D)
        )
        xt = xt_f.astype(BF16)
        xt8 = np.clip(xt_f * XS, -240, 240).astype(F8E4)
        in_maps.append(
            {
                "xt": xt,
                "xt8": xt8,
                "Wmp": wm_p,
                "Wp8": wp8_p,
                "bep": be_p,
                "Wgp": wg_p,
                "bgp": bg_p,
            }
        )
    return in_maps


def _run(inputs, trace=False):
    """Returns (y_full, BassKernelResults)."""
    from concourse.bass_utils import run_bass_kernel_spmd

    nc = _get_nc()
    in_maps = _pack_inputs(**inputs)
    res = run_bass_kernel_spmd(
        nc, in_maps, core_ids=list(range(NCORES)), trace=trace
    )
    y = np.concatenate(
        [res.results[i]["out"] for i in range(NCORES)], axis=0
    ).astype(np.float32)
    return y, res


def kernel(**inputs):
    y, _ = _run(inputs, trace=False)
    return y
